# revision 42
# baseline (speedup 1.0000x reference)
"""Causal multi-head attention (B=4, S=2048, D=1024, H=16, hd=64) on 8 TRN2
NeuronCores.

Sharding: core c = (batch b = c//2, head-group g = c%2). Each core computes
QKV projections for its 8 heads (Megatron column-split), causal attention,
and a partial out-projection (row-split); the host sums the two head-group
partials per batch and adds the bias.

On-device layout (bf16 compute, fp32 PSUM accumulation):
  xT  [1024, 2048]  x[b]^T           (din on partitions)
  qT/kT as [d_g, S] transposed tiles: head-pair t -> partitions
        [0:64] head 2t, [64:128] head 2t+1
  v   [k-tile 128, 8 heads, 65]: col 64 is ones (sumexp lands in the ctx^T
        psum row 64 for free during the attn*V matmul)
  scores^T psum tiles [k 128, 2 heads, q 512] (2 banks): head pair packed
        via PE row tiling (K=64 each), one exp / one mask-mul over both
  attn = exp(scores/8), no max-subtraction (|s|/8 <= ~3), causal handled by
        skipping k-tiles above the diagonal, restricting the q-range on
        diagonal tiles (s0 = dd*128), and a mask multiply for the boundary
  ctx^T accumulated in PSUM over k-tiles. Normalize: copy psum out (frees
        banks), round-trip the sumexp row through DRAM to a [128, 8] layout
        so the DVE iterative reciprocal runs on free-dim 8 (not 1024), then
        broadcast the reciprocal across partitions with a K=1 f32r
        ones-matmul and multiply.
"""

import numpy as np
import ml_dtypes

import concourse.bass as bass
import concourse.tile as tile
from concourse import bacc, mybir
from concourse.bass_utils import run_bass_kernel_spmd

P = 128          # partitions
S = 2048         # sequence length (one batch per core)
DIN = 1024       # model dim
DG = 512         # head-group width per core (8 heads x 64)
HD = 64          # head dim
NH = 8           # heads per core
QC = 512         # q-chunk (matmul free dim)
NQC = S // QC    # 4 q-chunks
NKT = S // P     # 16 k-tiles
KDT = DIN // P   # 8 din k-tiles
NHP = 4          # head pairs per core
F32 = mybir.dt.float32
F32R = mybir.dt.float32r
BF16 = mybir.dt.bfloat16
EXP = mybir.ActivationFunctionType.Exp

_CACHE = {}


def _emit(tc, d):
    nc = tc.nc
    with (
        nc.allow_low_precision(reason="bf16 attention pipeline"),
        tc.tile_pool(name="persist", bufs=1) as pp,
        tc.tile_pool(name="work", bufs=4) as wp,
        tc.tile_pool(name="psc", bufs=2, space="PSUM") as psc,
        tc.tile_pool(name="ppj", bufs=2, space="PSUM") as ppj,
        tc.tile_pool(name="pcx", bufs=1, space="PSUM") as pcx,
    ):
        # ---- persistent SBUF tiles ----
        xT = [pp.tile([P, S], BF16, tag=f"xT{k}", name=f"xT{k}") for k in range(KDT)]
        wq = [pp.tile([P, DG], BF16, tag=f"wq{k}", name=f"wq{k}") for k in range(KDT)]
        wk = [pp.tile([P, DG], BF16, tag=f"wk{k}", name=f"wk{k}") for k in range(KDT)]
        wv = [pp.tile([P, DG], BF16, tag=f"wv{k}", name=f"wv{k}") for k in range(KDT)]
        wo = [pp.tile([P, DIN], BF16, tag=f"wo{k}", name=f"wo{k}") for k in range(4)]
        qT = [pp.tile([P, S], BF16, tag=f"qT{t}", name=f"qT{t}") for t in range(NHP)]
        kT = [pp.tile([P, S], BF16, tag=f"kT{t}", name=f"kT{t}") for t in range(NHP)]
        vv = [pp.tile([P, NH, HD + 1], BF16, tag=f"v{m}", name=f"v{m}") for m in range(NKT)]
        cx = [pp.tile([P, S], BF16, tag=f"cx{t}", name=f"cx{t}") for t in range(NHP)]
        msk = pp.tile([P, 4, 2, QC], BF16, tag="msk", name="msk")

        # ---- input DMAs (xT + wv first: V projection starts the kernel;
        # xT in halves, split across sync/vector queues to parallelize
        # triggers; weights on scalar which is idle at start) ----
        for k in range(KDT):
            eng = nc.sync if k % 2 == 0 else nc.scalar
            eng.dma_start(xT[k][:, 0:2 * QC], d["xT"][k * P:(k + 1) * P, 0:2 * QC])
        for k in range(KDT):
            (nc.sync if k % 2 == 0 else nc.scalar).dma_start(
                wv[k][:], d["wvT"][k * P:(k + 1) * P, :]
            )
        for k in range(KDT):
            eng = nc.sync if k % 2 == 0 else nc.scalar
            eng.dma_start(
                xT[k][:, 2 * QC:S], d["xT"][k * P:(k + 1) * P, 2 * QC:S]
            )
        for k in range(KDT):
            nc.scalar.dma_start(wq[k][:], d["wqT"][k * P:(k + 1) * P, :])
            nc.scalar.dma_start(wk[k][:], d["wkT"][k * P:(k + 1) * P, :])
        for k in range(4):
            nc.scalar.dma_start(wo[k][:], d["woT"][k * P:(k + 1) * P, :])
        for dd in range(4):
            for h in range(2):
                nc.sync.dma_start(
                    msk[:, dd, h, :], d["masks"][:, dd * QC:(dd + 1) * QC]
                )


        def proj_v():
            for m in range(NKT):
                ps = ppj.tile([P, QC], F32, tag="pj", name="ps")
                for k in range(KDT):
                    nc.tensor.matmul(
                        ps[:],
                        xT[k][:, m * P:(m + 1) * P],
                        wv[k][:],
                        start=(k == 0),
                        stop=(k == KDT - 1),
                    )
                nc.vector.tensor_copy(
                    vv[m][:, :, 0:HD], ps[:].rearrange("p (h e) -> p h e", h=NH)
                )
                nc.vector.memset(vv[m][:, :, HD:HD + 1], 1.0)

        def proj_qk_chain(t, w, s):
            wt, dst = ((wq, qT), (wk, kT))[w]
            ps = ppj.tile([P, QC], F32, tag="pj", name="ps")
            for k in range(KDT):
                nc.tensor.matmul(
                    ps[:],
                    wt[k][:, t * P:(t + 1) * P],
                    xT[k][:, s * QC:(s + 1) * QC],
                    start=(k == 0),
                    stop=(k == KDT - 1),
                )
            nc.vector.tensor_copy(dst[t][:, s * QC:(s + 1) * QC], ps[:])

        def proj_qk(t):
            for w in range(2):
                for s in range(NQC):
                    proj_qk_chain(t, w, s)

        def attn_chunk(hp, s):
                ci = hp * NQC + s
                nkt = 4 * (s + 1)  # causal: k-tiles 0..nkt-1
                cps = pcx.tile([HD + 1, 2, QC], F32, tag="cx", name="cps")
                for k in range(nkt):
                    dd = k - 4 * s
                    s0 = max(dd, 0) * P  # causal q-range restriction
                    sps = psc.tile([P, 2, QC], F32, tag="sc", name="sps")
                    nc.tensor.matmul(
                        sps[:, 0, s0:],
                        kT[hp][0:HD, k * P:(k + 1) * P],
                        qT[hp][0:HD, s * QC + s0:(s + 1) * QC],
                        start=True, stop=True,
                    )
                    nc.tensor.matmul(
                        sps[:, 1, s0:],
                        kT[hp][HD:P, k * P:(k + 1) * P],
                        qT[hp][HD:P, s * QC + s0:(s + 1) * QC],
                        start=True, stop=True,
                    )
                    a = wp.tile([P, 2, QC], BF16, tag="a", name="a", bufs=6)
                    nc.scalar.activation(
                        a[:, :, s0:], sps[:, :, s0:], EXP, scale=0.125
                    )
                    if dd >= 0:
                        # only columns [s0, s0+128) straddle the diagonal;
                        # everything past them is fully valid
                        nc.vector.tensor_mul(
                            a[:, :, s0:s0 + P], a[:, :, s0:s0 + P],
                            msk[:, dd, :, s0:s0 + P],
                        )
                    nc.tensor.matmul(
                        cps[:, 0, s0:], vv[k][:, 2 * hp, :], a[:, 0, s0:],
                        start=(k == 0), stop=(k == nkt - 1),
                    )
                    nc.tensor.matmul(
                        cps[:, 1, s0:], vv[k][:, 2 * hp + 1, :], a[:, 1, s0:],
                        start=(k == 0), stop=(k == nkt - 1),
                    )
                # normalize: rows 0:64 are ctx^T, row 64 is sumexp
                cb = wp.tile([HD + 1, 2, QC], F32, tag="cb", name="cb", bufs=2)
                nc.vector.tensor_copy(cb[:], cps[:])
                # reciprocal of the [1, 1024] sumexp row with free-dim 8:
                # reshape to [128, 8] via SBUF-SBUF DMA so the DVE iterative
                # divide (8 cyc/elem along free dim) runs on free-dim 8
                zt = wp.tile([P, 8], F32, tag="zt", name="zt")
                nc.sync.dma_start(zt[:], cb[HD:HD + 1, :, :])
                rt = wp.tile([P, 8], F32, tag="rt", name="rt")
                nc.vector.reciprocal(rt[:], zt[:])
                rc = wp.tile([P, 2, QC], F32, tag="rc", name="rc")
                nc.sync.dma_start(rc[0:1, :, :], rt[:])
                bs = wp.tile([HD, 2, QC], F32, tag="bs", name="bs", bufs=2)
                nc.gpsimd.partition_broadcast(bs[:], rc[0:1, :, :])
                nc.vector.tensor_mul(
                    cx[hp][0:HD, s * QC:(s + 1) * QC],
                    cb[0:HD, 0, :], bs[:, 0, :],
                )
                cxs = wp.tile([HD, QC], BF16, tag="cxs", name="cxs")
                nc.vector.tensor_mul(cxs[:], cb[0:HD, 1, :], bs[:, 1, :])
                # shift partitions 0:64 -> 64:128 via SBUF DMA
                nc.sync.dma_start(
                    cx[hp][HD:P, s * QC:(s + 1) * QC], cxs[:]
                )

        def out_proj_s(s):
            for o in range(DIN // P):
                ps = ppj.tile([P, QC], F32, tag="pj", name="ps")
                for k in range(4):
                    nc.tensor.matmul(
                        ps[:],
                        wo[k][:, o * P:(o + 1) * P],
                        cx[k][:, s * QC:(s + 1) * QC],
                        start=(k == 0), stop=(k == 3),
                    )
                ob = wp.tile([P, QC], F32, tag="ob", name="ob")
                nc.scalar.copy(ob[:], ps[:])
                nc.sync.dma_start(
                    d["outT"][o * P:(o + 1) * P, s * QC:(s + 1) * QC], ob[:]
                )

        proj_v()
        proj_qk(0)
        for t in range(NHP):
            for s in range(NQC):
                attn_chunk(t, s)
                if t + 1 < NHP:
                    # overlap next head-pair's Q/K projection with this
                    # head-pair's (ACT-bound) attention
                    proj_qk_chain(t + 1, 0, s)
                    proj_qk_chain(t + 1, 1, s)
        for s in range(NQC):
            out_proj_s(s)


def _build():
    if "nc" in _CACHE:
        return _CACHE["nc"]
    nc = bacc.Bacc("TRN2", target_bir_lowering=False, debug=False, num_devices=8)
    d = {
        "xT": nc.dram_tensor("xT", [DIN, S], BF16, kind="ExternalInput").ap(),
        "wqT": nc.dram_tensor("wqT", [DIN, DG], BF16, kind="ExternalInput").ap(),
        "wkT": nc.dram_tensor("wkT", [DIN, DG], BF16, kind="ExternalInput").ap(),
        "wvT": nc.dram_tensor("wvT", [DIN, DG], BF16, kind="ExternalInput").ap(),
        "woT": nc.dram_tensor("woT", [DG, DIN], BF16, kind="ExternalInput").ap(),
        "masks": nc.dram_tensor("masks", [P, 4 * QC], BF16, kind="ExternalInput").ap(),
        "outT": nc.dram_tensor("outT", [DIN, S], F32, kind="ExternalOutput").ap(),
    }
    with tile.TileContext(nc, pool_alloc_mode="queue") as tc:
        _emit(tc, d)
    nc.compile()
    _CACHE["nc"] = nc
    return nc


def _masks_np():
    r = np.arange(P)[:, None]
    j = np.arange(QC)[None, :]
    return np.concatenate(
        [(j >= r + dd * P).astype(ml_dtypes.bfloat16) for dd in range(4)], axis=1
    )


def kernel(x, Wq, Wk, Wv, Wo, bo, _run_kwargs=None, _return_res=False):
    x = np.asarray(x)
    Wq, Wk, Wv, Wo, bo = (np.asarray(a) for a in (Wq, Wk, Wv, Wo, bo))
    B = x.shape[0]
    nc = _build()

    def b16(a):
        return np.ascontiguousarray(a).astype(ml_dtypes.bfloat16)

    masks = _masks_np()
    in_maps = []
    for c in range(8):
        b, g = divmod(c, 2)
        in_maps.append({
            "xT": b16(x[b].T),
            "wqT": b16(Wq[g * DG:(g + 1) * DG, :].T),
            "wkT": b16(Wk[g * DG:(g + 1) * DG, :].T),
            "wvT": b16(Wv[g * DG:(g + 1) * DG, :].T),
            "woT": b16(Wo[:, g * DG:(g + 1) * DG].T),
            "masks": masks,
        })

    res = run_bass_kernel_spmd(nc, in_maps, list(range(8)), **(_run_kwargs or {}))
    out = np.empty((B, S, DIN), np.float32)
    for b in range(B):
        p = res.results[2 * b]["outT"] + res.results[2 * b + 1]["outT"]
        out[b] = p.T + bo.astype(np.float32)
    if _return_res:
        return out, res
    return out


# revision 43
# speedup vs baseline: 1.0016x; 1.0016x over previous
"""Causal multi-head attention (B=4, S=2048, D=1024, H=16, hd=64) on 8 TRN2
NeuronCores.

Sharding: core c = (batch b = c//2, head-group g = c%2). Each core computes
QKV projections for its 8 heads (Megatron column-split), causal attention,
and a partial out-projection (row-split); the host sums the two head-group
partials per batch and adds the bias.

On-device layout (bf16 compute, fp32 PSUM accumulation):
  xT  [1024, 2048]  x[b]^T           (din on partitions)
  qT/kT as [d_g, S] transposed tiles: head-pair t -> partitions
        [0:64] head 2t, [64:128] head 2t+1
  v   [k-tile 128, 8 heads, 65]: col 64 is ones (sumexp lands in the ctx^T
        psum row 64 for free during the attn*V matmul)
  scores^T psum tiles [k 128, 2 heads, q 512] (2 banks): head pair packed
        via PE row tiling (K=64 each), one exp / one mask-mul over both
  attn = exp(scores/8), no max-subtraction (|s|/8 <= ~3), causal handled by
        skipping k-tiles above the diagonal, restricting the q-range on
        diagonal tiles (s0 = dd*128), and a mask multiply for the boundary
  ctx^T accumulated in PSUM over k-tiles. Normalize: copy psum out (frees
        banks), round-trip the sumexp row through DRAM to a [128, 8] layout
        so the DVE iterative reciprocal runs on free-dim 8 (not 1024), then
        broadcast the reciprocal across partitions with a K=1 f32r
        ones-matmul and multiply.
"""

import numpy as np
import ml_dtypes

import concourse.bass as bass
import concourse.tile as tile
from concourse import bacc, mybir
from concourse.bass_utils import run_bass_kernel_spmd

P = 128          # partitions
S = 2048         # sequence length (one batch per core)
DIN = 1024       # model dim
DG = 512         # head-group width per core (8 heads x 64)
HD = 64          # head dim
NH = 8           # heads per core
QC = 512         # q-chunk (matmul free dim)
NQC = S // QC    # 4 q-chunks
NKT = S // P     # 16 k-tiles
KDT = DIN // P   # 8 din k-tiles
NHP = 4          # head pairs per core
F32 = mybir.dt.float32
F32R = mybir.dt.float32r
BF16 = mybir.dt.bfloat16
EXP = mybir.ActivationFunctionType.Exp

_CACHE = {}


def _emit(tc, d):
    nc = tc.nc
    with (
        nc.allow_low_precision(reason="bf16 attention pipeline"),
        tc.tile_pool(name="persist", bufs=1) as pp,
        tc.tile_pool(name="work", bufs=4) as wp,
        tc.tile_pool(name="psc", bufs=2, space="PSUM") as psc,
        tc.tile_pool(name="ppj", bufs=2, space="PSUM") as ppj,
        tc.tile_pool(name="pcx", bufs=1, space="PSUM") as pcx,
    ):
        # ---- persistent SBUF tiles ----
        xT = [pp.tile([P, S], BF16, tag=f"xT{k}", name=f"xT{k}") for k in range(KDT)]
        wq = [pp.tile([P, DG], BF16, tag=f"wq{k}", name=f"wq{k}") for k in range(KDT)]
        wk = [pp.tile([P, DG], BF16, tag=f"wk{k}", name=f"wk{k}") for k in range(KDT)]
        wv = [pp.tile([P, DG], BF16, tag=f"wv{k}", name=f"wv{k}") for k in range(KDT)]
        wo = [pp.tile([P, DIN], BF16, tag=f"wo{k}", name=f"wo{k}") for k in range(4)]
        qT = [pp.tile([P, S], BF16, tag=f"qT{t}", name=f"qT{t}") for t in range(NHP)]
        kT = [pp.tile([P, S], BF16, tag=f"kT{t}", name=f"kT{t}") for t in range(NHP)]
        vv = [pp.tile([P, NH, HD + 1], BF16, tag=f"v{m}", name=f"v{m}") for m in range(NKT)]
        cx = [pp.tile([P, S], BF16, tag=f"cx{t}", name=f"cx{t}") for t in range(NHP)]
        msk = pp.tile([P, 4, 2, QC], BF16, tag="msk", name="msk")

        # ---- input DMAs (xT + wv first: V projection starts the kernel;
        # xT in halves, split across sync/vector queues to parallelize
        # triggers; weights on scalar which is idle at start) ----
        for k in range(KDT):
            eng = nc.sync if k % 2 == 0 else nc.scalar
            eng.dma_start(xT[k][:, 0:2 * QC], d["xT"][k * P:(k + 1) * P, 0:2 * QC])
        for k in range(KDT):
            (nc.sync if k % 2 == 0 else nc.scalar).dma_start(
                wv[k][:], d["wvT"][k * P:(k + 1) * P, :]
            )
        for k in range(KDT):
            eng = nc.sync if k % 2 == 0 else nc.scalar
            eng.dma_start(
                xT[k][:, 2 * QC:S], d["xT"][k * P:(k + 1) * P, 2 * QC:S]
            )
        for k in range(KDT):
            nc.scalar.dma_start(wq[k][:], d["wqT"][k * P:(k + 1) * P, :])
            nc.scalar.dma_start(wk[k][:], d["wkT"][k * P:(k + 1) * P, :])
        for k in range(4):
            nc.scalar.dma_start(wo[k][:], d["woT"][k * P:(k + 1) * P, :])
        for dd in range(4):
            for h in range(2):
                nc.sync.dma_start(
                    msk[:, dd, h, :], d["masks"][:, dd * QC:(dd + 1) * QC]
                )


        def proj_v():
            for m in range(NKT):
                ps = ppj.tile([P, QC], F32, tag="pj", name="ps")
                for k in range(KDT):
                    nc.tensor.matmul(
                        ps[:],
                        xT[k][:, m * P:(m + 1) * P],
                        wv[k][:],
                        start=(k == 0),
                        stop=(k == KDT - 1),
                    )
                nc.vector.tensor_copy(
                    vv[m][:, :, 0:HD], ps[:].rearrange("p (h e) -> p h e", h=NH)
                )
                nc.vector.memset(vv[m][:, :, HD:HD + 1], 1.0)

        def proj_qk_chain(t, w, s):
            wt, dst = ((wq, qT), (wk, kT))[w]
            ps = ppj.tile([P, QC], F32, tag="pj", name="ps")
            for k in range(KDT):
                nc.tensor.matmul(
                    ps[:],
                    wt[k][:, t * P:(t + 1) * P],
                    xT[k][:, s * QC:(s + 1) * QC],
                    start=(k == 0),
                    stop=(k == KDT - 1),
                )
            nc.vector.tensor_copy(dst[t][:, s * QC:(s + 1) * QC], ps[:])

        def proj_qk(t):
            for w in range(2):
                for s in range(NQC):
                    proj_qk_chain(t, w, s)

        def attn_chunk(hp, s):
                ci = hp * NQC + s
                nkt = 4 * (s + 1)  # causal: k-tiles 0..nkt-1
                cps = pcx.tile([HD + 1, 2, QC], F32, tag="cx", name="cps")
                for k in range(nkt):
                    dd = k - 4 * s
                    s0 = max(dd, 0) * P  # causal q-range restriction
                    sps = psc.tile([P, 2, QC], F32, tag="sc", name="sps")
                    nc.tensor.matmul(
                        sps[:, 0, s0:],
                        kT[hp][0:HD, k * P:(k + 1) * P],
                        qT[hp][0:HD, s * QC + s0:(s + 1) * QC],
                        start=True, stop=True,
                    )
                    nc.tensor.matmul(
                        sps[:, 1, s0:],
                        kT[hp][HD:P, k * P:(k + 1) * P],
                        qT[hp][HD:P, s * QC + s0:(s + 1) * QC],
                        start=True, stop=True,
                    )
                    a = wp.tile([P, 2, QC], BF16, tag="a", name="a", bufs=6)
                    nc.scalar.activation(
                        a[:, :, s0:], sps[:, :, s0:], EXP, scale=0.125
                    )
                    if dd >= 0:
                        # only columns [s0, s0+128) straddle the diagonal;
                        # everything past them is fully valid
                        nc.vector.tensor_mul(
                            a[:, :, s0:s0 + P], a[:, :, s0:s0 + P],
                            msk[:, dd, :, s0:s0 + P],
                        )
                    nc.tensor.matmul(
                        cps[:, 0, s0:], vv[k][:, 2 * hp, :], a[:, 0, s0:],
                        start=(k == 0), stop=(k == nkt - 1),
                    )
                    nc.tensor.matmul(
                        cps[:, 1, s0:], vv[k][:, 2 * hp + 1, :], a[:, 1, s0:],
                        start=(k == 0), stop=(k == nkt - 1),
                    )
                # normalize: rows 0:64 are ctx^T, row 64 is sumexp
                cb = wp.tile([HD + 1, 2, QC], F32, tag="cb", name="cb", bufs=2)
                nc.vector.tensor_copy(cb[:], cps[:])
                # reciprocal of the [1, 1024] sumexp row with free-dim 8:
                # reshape to [128, 8] via SBUF-SBUF DMA so the DVE iterative
                # divide (8 cyc/elem along free dim) runs on free-dim 8
                zt = wp.tile([P, 8], F32, tag="zt", name="zt")
                nc.sync.dma_start(zt[:], cb[HD:HD + 1, :, :])
                rt = wp.tile([P, 8], F32, tag="rt", name="rt")
                nc.vector.reciprocal(rt[:], zt[:])
                rc = wp.tile([P, 2, QC], F32, tag="rc", name="rc")
                nc.sync.dma_start(rc[0:1, :, :], rt[:])
                bs = wp.tile([HD, 2, QC], F32, tag="bs", name="bs", bufs=2)
                nc.gpsimd.partition_broadcast(bs[:], rc[0:1, :, :])
                nc.vector.tensor_mul(
                    cx[hp][0:HD, s * QC:(s + 1) * QC],
                    cb[0:HD, 0, :], bs[:, 0, :],
                )
                cxs = wp.tile([HD, QC], BF16, tag="cxs", name="cxs")
                nc.vector.tensor_mul(cxs[:], cb[0:HD, 1, :], bs[:, 1, :])
                # shift partitions 0:64 -> 64:128 via SBUF DMA
                nc.sync.dma_start(
                    cx[hp][HD:P, s * QC:(s + 1) * QC], cxs[:]
                )

        def out_proj_s(s):
            for o in range(DIN // P):
                ps = ppj.tile([P, QC], F32, tag="pj", name="ps")
                for k in range(4):
                    nc.tensor.matmul(
                        ps[:],
                        wo[k][:, o * P:(o + 1) * P],
                        cx[k][:, s * QC:(s + 1) * QC],
                        start=(k == 0), stop=(k == 3),
                    )
                ob = wp.tile([P, QC], F32, tag="ob", name="ob")
                nc.scalar.copy(ob[:], ps[:])
                nc.sync.dma_start(
                    d["outT"][o * P:(o + 1) * P, s * QC:(s + 1) * QC], ob[:]
                )

        proj_v()
        proj_qk(0)
        for t in range(NHP):
            for s in range(NQC):
                attn_chunk(t, s)
                if t + 1 < NHP:
                    # overlap next head-pair's Q/K projection with this
                    # head-pair's (ACT-bound) attention
                    proj_qk_chain(t + 1, 0, s)
                    proj_qk_chain(t + 1, 1, s)
        for s in range(NQC):
            out_proj_s(s)


def _build():
    if "nc" in _CACHE:
        return _CACHE["nc"]
    nc = bacc.Bacc("TRN2", target_bir_lowering=False, debug=False, num_devices=8)
    d = {
        "xT": nc.dram_tensor("xT", [DIN, S], BF16, kind="ExternalInput").ap(),
        "wqT": nc.dram_tensor("wqT", [DIN, DG], BF16, kind="ExternalInput").ap(),
        "wkT": nc.dram_tensor("wkT", [DIN, DG], BF16, kind="ExternalInput").ap(),
        "wvT": nc.dram_tensor("wvT", [DIN, DG], BF16, kind="ExternalInput").ap(),
        "woT": nc.dram_tensor("woT", [DG, DIN], BF16, kind="ExternalInput").ap(),
        "masks": nc.dram_tensor("masks", [P, 4 * QC], BF16, kind="ExternalInput").ap(),
        "outT": nc.dram_tensor("outT", [DIN, S], F32, kind="ExternalOutput").ap(),
    }
    with tile.TileContext(nc) as tc:
        _emit(tc, d)
    nc.compile()
    _CACHE["nc"] = nc
    return nc


def _masks_np():
    r = np.arange(P)[:, None]
    j = np.arange(QC)[None, :]
    return np.concatenate(
        [(j >= r + dd * P).astype(ml_dtypes.bfloat16) for dd in range(4)], axis=1
    )


def kernel(x, Wq, Wk, Wv, Wo, bo, _run_kwargs=None, _return_res=False):
    x = np.asarray(x)
    Wq, Wk, Wv, Wo, bo = (np.asarray(a) for a in (Wq, Wk, Wv, Wo, bo))
    B = x.shape[0]
    nc = _build()

    def b16(a):
        return np.ascontiguousarray(a).astype(ml_dtypes.bfloat16)

    masks = _masks_np()
    in_maps = []
    for c in range(8):
        b, g = divmod(c, 2)
        in_maps.append({
            "xT": b16(x[b].T),
            "wqT": b16(Wq[g * DG:(g + 1) * DG, :].T),
            "wkT": b16(Wk[g * DG:(g + 1) * DG, :].T),
            "wvT": b16(Wv[g * DG:(g + 1) * DG, :].T),
            "woT": b16(Wo[:, g * DG:(g + 1) * DG].T),
            "masks": masks,
        })

    res = run_bass_kernel_spmd(nc, in_maps, list(range(8)), **(_run_kwargs or {}))
    out = np.empty((B, S, DIN), np.float32)
    for b in range(B):
        p = res.results[2 * b]["outT"] + res.results[2 * b + 1]["outT"]
        out[b] = p.T + bo.astype(np.float32)
    if _return_res:
        return out, res
    return out
